# revision 15
# baseline (speedup 1.0000x reference)
"""CARAFE-Downsample Trainium2 kernel (8 NeuronCores, data-parallel over batch).

Problem (hardcoded shapes): x [8, 256, 128, 128] f32; 1x1-conv compressor ->
cx [8, 64, 128, 128]; 3x3 stride-2 conv encoder -> mask [8, 25, 64, 64];
softmax(mask * exp(p)) over the 25 taps; 5x5 stride-2 weighted reassembly of x
-> out [8, 256, 64, 64].

Strategy:
 - one sample per core (B == n_cores == 8).
 - Pixel-block layout: output block k (k in 0..31) holds the 128 output pixels
   {(h', w') : h' in {k, k+32}, w' in 0..63} on the 128 SBUF partitions
   (p = half*64 + w').  With this (k, k+32) row pairing every 5x5 tap of the
   reassembly is a single full-width fused-MAC with the softmax weight as a
   per-partition scalar: tap (i,j) reads host-prepared "slab" (oh, j) at block
   row k + dh (oh = (i-2)%2, dh = (i-2-oh)//2); slabs carry x pre-gathered
   (stride-2 cols, row-parity split) with zero padding baked in (34 block
   rows kk = -1..32).  No transposes of x, no partition shifts, no border
   fix-ups on device.
 - reassembly is split across three engines per block: DVE runs a
   scalar_tensor_tensor chain, ACT produces weighted copies (activation
   Copy with per-partition scale), GPSIMD accumulates those and combines.
 - mask path (compressor + encoder) runs in bf16 on the TensorEngine
   (rel err contribution ~3e-4); softmax weights and the reassembly stay
   f32.  Encoder computes mask [25, 512]-chunks (3x3/s2 conv as 9 matmul
   accumulates with a 2D strided moving operand), then PE-transposes
   [25, 64] slices into the block layout.  exp(power_p) is folded into the
   encoder weights on host; conv biases are K=1 rank-1 matmul accumulates.
 - output is returned in [32 blocks, 128, 256] layout, host restores NCHW.
"""

import numpy as np
import ml_dtypes

import concourse.bass as bass
import concourse.bacc as bacc
import concourse.tile as tile
from concourse import mybir
from concourse.bass_utils import run_bass_kernel_spmd

# -- problem constants (hardcoded per spec) ---------------------------------
B, C, H, W = 8, 256, 128, 128
CC = 64           # compressed channels
KK = 5            # CARAFE window
HP = WP = 64      # output spatial
NB = 32           # pixel blocks per sample
NCORES = 8

# dtype knobs
X_DTYPE = "bf16"    # slabs ("f32" safe / "bf16" fast; PE reassembly needs bf16)
MASK_DTYPE = "bf16"  # compressor/encoder path (error contribution ~3e-4)

# reassembly engine: "pe" = diag-matmul accumulation on TensorE (bf16 slabs,
# f32 psum accumulate, ~2.8e-3 rel err); "dve" = f32 fused-MAC chains split
# across DVE/ACT/GPSIMD (~4e-4 rel err)
REASM = "pe"

# dve-mode tap split: 25 taps total; N_ACT go ACT(product)+GPSIMD(add)
N_ACT = 7
# pe-mode: number of taps run as a DVE bf16 chain instead of PE diag-matmuls
N_DVE = 8

_DTM = {"f32": mybir.dt.float32, "bf16": mybir.dt.bfloat16}
_NPM = {"f32": np.float32, "bf16": ml_dtypes.bfloat16}
DTX, DTK = _DTM[X_DTYPE], _DTM[MASK_DTYPE]
NPX, NPK = _NPM[X_DTYPE], _NPM[MASK_DTYPE]
F32 = mybir.dt.float32

# tap -> (slab index, block-row offset). slab sl = oh*5 + j holds x rows of
# parity oh, cols (j-2)+2*w'' (zero padded), block rows kk = -1..32.
def _tap_table():
    taps = []
    for i in range(KK):
        oh = (i - 2) % 2
        dh = (i - 2 - oh) // 2
        for j in range(KK):
            taps.append((i * 5 + j, oh * 5 + j, dh))
    return taps

_TAPS = _tap_table()


def _build_nc():
    nc = bacc.Bacc(None, target_bir_lowering=False, debug=False)

    xc_d = nc.declare_dram_parameter("xc", [2, 128, H * W], DTK, isOutput=False)
    sl_d = nc.declare_dram_parameter("slabs", [34, 128, 10, C], DTX, isOutput=False)
    wc_d = nc.declare_dram_parameter("wc", [2, 128, CC], DTK, isOutput=False)
    bc_d = nc.declare_dram_parameter("bc", [CC, 1], F32, isOutput=False)
    wt_d = nc.declare_dram_parameter("wt", [CC, 9, 25], DTK, isOutput=False)
    be_d = nc.declare_dram_parameter("be", [25, 1], F32, isOutput=False)
    id_d = nc.declare_dram_parameter("idn", [25, 25], DTK, isOutput=False)
    i128_d = nc.declare_dram_parameter("i128", [128, 128], DTX, isOutput=False)
    out_d = nc.declare_dram_parameter("out", [NB, 128, C], F32, isOutput=True)

    CXW = 130  # padded cx row length; cx_pad[c, r*130 + col], r/col offset by 1

    with tile.TileContext(nc) as tc:
        with (
            tc.tile_pool(name="consts", bufs=1) as consts,
            tc.tile_pool(name="xcin", bufs=3) as xcin,
            tc.tile_pool(name="cx", bufs=1) as cxpool,
            tc.tile_pool(name="psA", bufs=2, space="PSUM") as psA,
            tc.tile_pool(name="psM", bufs=2, space="PSUM") as psM,
            tc.tile_pool(name="psT", bufs=1, space="PSUM") as psT,
            tc.tile_pool(name="psO", bufs=3, space="PSUM") as psO,
            tc.tile_pool(name="soft", bufs=6) as soft,
            tc.tile_pool(name="wmask", bufs=8) as wmask,
            tc.tile_pool(name="slab", bufs=7) as slabp,
            tc.tile_pool(name="accp", bufs=4) as accp,
            tc.tile_pool(name="prod", bufs=8) as prodp,
        ):
            # ---- constants / weights ----
            wc_sb = consts.tile([128, 2, CC], DTK)
            nc.sync.dma_start(out=wc_sb, in_=wc_d[:, :, :].rearrange("c p m -> p c m"))
            wt_sb = consts.tile([CC, 9, 25], DTK)
            nc.sync.dma_start(out=wt_sb, in_=wt_d[:, :, :])
            bc_sb = consts.tile([CC, 1], F32)
            nc.sync.dma_start(out=bc_sb, in_=bc_d[:, :])
            be_sb = consts.tile([25, 1], F32)
            nc.sync.dma_start(out=be_sb, in_=be_d[:, :])
            id_sb = consts.tile([25, 25], DTK)
            nc.sync.dma_start(out=id_sb, in_=id_d[:, :])
            i128_sb = consts.tile([128, 128], DTX)
            nc.sync.dma_start(out=i128_sb, in_=i128_d[:, :])

            # ---- cx_pad (compressor output, 1-px zero ring, flat layout) ----
            cx_pad = cxpool.tile([CC, CXW * CXW], DTK)
            cp = cx_pad[:, :]
            # zero pad row 0 / col 0 (the only pad the encoder reads) via ACT
            # so cx_pad has a single writer engine (keeps PE matmul waits at 1)
            zrow = consts.tile([CC, CXW], DTK)
            nc.vector.memset(zrow, 0.0)
            nc.scalar.copy(out=cp[:, 0:CXW], in_=zrow[:, :])
            nc.scalar.copy(
                out=bass.AP(tensor=cp.tensor, offset=cp.offset + CXW,
                            ap=[cp.ap[0], [CXW, 129], [1, 1]]),
                in_=zrow[:, 0:129],
            )

            # all-engine sync after const loads: keeps every later PE matmul
            # at <=1 sync wait (PE LDWEIGHTS has a single wait slot)
            tc.strict_bb_all_engine_barrier()

            # ---- phase A: compressor 1x1 conv (PE, bf16) ----
            for j in range(32):
                xt = xcin.tile([128, 2, 512], DTK)
                nc.sync.dma_start(
                    out=xt,
                    in_=xc_d[:, :, j * 512:(j + 1) * 512].rearrange("c p n -> p c n"),
                )
                pm = psA.tile([CC, 512], F32)
                nc.tensor.matmul(pm, lhsT=wc_sb[:, 0, :], rhs=xt[:, 0, :],
                                 start=True, stop=False)
                nc.tensor.matmul(pm, lhsT=wc_sb[:, 1, :], rhs=xt[:, 1, :],
                                 start=False, stop=True)
                # rows 4j..4j+3 of cx -> cx_pad interior (offset by 1 row/col)
                dst = bass.AP(tensor=cp.tensor,
                              offset=cp.offset + (4 * j + 1) * CXW + 1,
                              ap=[cp.ap[0], [CXW, 4], [1, 128]])
                nc.scalar.activation(out=dst,
                                     in_=pm[:, :].rearrange("p (r n) -> p r n", n=128),
                                     func=mybir.ActivationFunctionType.Identity,
                                     bias=bc_sb[:, :])

            # ---- phase B: encoder 3x3/s2 conv -> m_all [25, 4096] (bf16) ----
            m_all = cxpool.tile([25, HP * WP], DTK)
            for j2 in range(8):
                pmM = psM.tile([25, 512], F32)
                ti = 0
                for di in range(3):
                    for dj in range(3):
                        # output pixels h' = 8*j2 + r (r 0..7), w' 0..63;
                        # reads cx_pad row 2h'+di, col 2w'+dj
                        rhs = bass.AP(
                            tensor=cp.tensor,
                            offset=cp.offset + (16 * j2 + di) * CXW + dj,
                            ap=[cp.ap[0], [2 * CXW, 8], [2, 64]],
                        )
                        nc.tensor.matmul(pmM, lhsT=wt_sb[:, ti, :], rhs=rhs,
                                         start=(ti == 0), stop=(ti == 8))
                        ti += 1
                nc.scalar.activation(out=m_all[:, j2 * 512:(j2 + 1) * 512],
                                     in_=pmM,
                                     func=mybir.ActivationFunctionType.Identity,
                                     bias=be_sb[:, :])

            # ---- phase C: per block: transpose + exp + softmax weights ----
            w_blocks = []
            for k in range(NB):
                e_k = soft.tile([128, 25], F32)
                for half in range(2):
                    hcol = (k + 32 * half) * 64
                    pmT = psT.tile([64, 25], DTK)
                    nc.tensor.transpose(pmT, m_all[:, hcol:hcol + 64], id_sb[:, :])
                    nc.scalar.activation(out=e_k[half * 64:(half + 1) * 64, :],
                                         in_=pmT,
                                         func=mybir.ActivationFunctionType.Exp)
                r_k = soft.tile([128, 1], F32)
                nc.vector.reduce_sum(out=r_k, in_=e_k, axis=mybir.AxisListType.X)
                nc.vector.reciprocal(out=r_k, in_=r_k)
                w_k = wmask.tile([128, 25], F32)
                rb = bass.AP(tensor=r_k.tensor, offset=r_k.offset,
                             ap=[r_k.ap[0], [0, 25]])
                nc.vector.tensor_tensor(out=w_k, in0=e_k, in1=rb,
                                        op=mybir.AluOpType.mult)
                w_blocks.append(w_k)

            # ---- phase D: reassembly, 3-engine split per block ----
            slab_tiles = []
            for kk in range(34):
                st = slabp.tile([128, 10, C], DTX)
                nc.sync.dma_start(out=st, in_=sl_d[kk, :, :, :])
                slab_tiles.append(st)

            tapmap = {t: (sl, dh) for (t, sl, dh) in _TAPS}
            if REASM == "pe":
                # N_DVE taps run as a DVE bf16 fused-MAC chain; the rest are
                # diag-matmul accumulates on PE: psum += diag(w_t) @ slab_slice
                all_taps = sorted(tapmap)
                dve_taps = all_taps[:N_DVE]
                pe_taps = all_taps[N_DVE:]
                with tc.tile_pool(name="diag", bufs=6) as diagp:
                    for k in range(NB):
                        w_k = w_blocks[k]
                        po = psO.tile([128, C], F32)
                        for n, t in enumerate(pe_taps):
                            sl, dh = tapmap[t]
                            D = diagp.tile([128, 128], DTX, name=f"D_{k}_{t}",
                                           tag="diag")
                            nc.vector.tensor_scalar(out=D, in0=i128_sb,
                                                    scalar1=w_k[:, t:t + 1],
                                                    scalar2=None,
                                                    op0=mybir.AluOpType.mult)
                            nc.tensor.matmul(po, lhsT=D,
                                             rhs=slab_tiles[k + dh + 1][:, sl, :],
                                             start=(n == 0),
                                             stop=(n == len(pe_taps) - 1))
                        # two short bf16 chains bound the accumulator
                        # rounding depth; combined in f32
                        accs = [accp.tile([128, C], DTX, name=f"acc{i}_{k}",
                                          tag=f"acc{i}") for i in range(2)]
                        for n, t in enumerate(dve_taps):
                            sl, dh = tapmap[t]
                            src_ = slab_tiles[k + dh + 1][:, sl, :]
                            sc = w_k[:, t:t + 1]
                            a = accs[n % 2]
                            if n < 2:
                                nc.vector.tensor_scalar(out=a, in0=src_,
                                                        scalar1=sc, scalar2=None,
                                                        op0=mybir.AluOpType.mult)
                            else:
                                nc.vector.scalar_tensor_tensor(
                                    out=a, in0=src_, scalar=sc, in1=a,
                                    op0=mybir.AluOpType.mult,
                                    op1=mybir.AluOpType.add)
                        fin = accp.tile([128, C], F32, tag="fin")
                        nc.vector.scalar_tensor_tensor(
                            out=fin, in0=accs[0], scalar=1.0, in1=accs[1],
                            op0=mybir.AluOpType.mult, op1=mybir.AluOpType.add)
                        nc.vector.tensor_tensor(out=fin, in0=fin, in1=po,
                                                op=mybir.AluOpType.add)
                        nc.sync.dma_start(out=out_d[k, :, :], in_=fin)
            else:
                # center tap (dh=0) first on DVE to initialize its accumulator;
                # N_ACT taps go to ACT(product) + GPSIMD(accumulate)
                dve_order = [12] + [t for t in range(25) if t != 12][N_ACT:]
                act_order = [t for t in range(25) if t != 12][:N_ACT]
                for k in range(NB):
                    w_k = w_blocks[k]
                    acc = accp.tile([128, C], DTX)
                    fin = accp.tile([128, C], F32, tag="fin")
                    acc2 = accp.tile([128, C], F32, tag="acc2")
                    prods = []
                    for t in act_order:
                        sl, dh = tapmap[t]
                        p_t = prodp.tile([128, C], F32, name=f"p_{k}_{t}", tag="prod")
                        nc.scalar.activation(out=p_t,
                                             in_=slab_tiles[k + dh + 1][:, sl, :],
                                             func=mybir.ActivationFunctionType.Copy,
                                             scale=w_k[:, t:t + 1])
                        prods.append(p_t)
                    nc.gpsimd.tensor_add(acc2, prods[0], prods[1])
                    for p_t in prods[2:]:
                        nc.gpsimd.tensor_add(acc2, acc2, p_t)
                    for n, t in enumerate(dve_order):
                        sl, dh = tapmap[t]
                        src_ = slab_tiles[k + dh + 1][:, sl, :]
                        sc = w_k[:, t:t + 1]
                        if n == 0:
                            nc.vector.tensor_scalar(out=acc, in0=src_, scalar1=sc,
                                                    scalar2=None,
                                                    op0=mybir.AluOpType.mult)
                        else:
                            nc.vector.scalar_tensor_tensor(
                                out=acc, in0=src_, scalar=sc, in1=acc,
                                op0=mybir.AluOpType.mult, op1=mybir.AluOpType.add)
                    nc.gpsimd.tensor_add(fin, acc, acc2)
                    nc.sync.dma_start(out=out_d[k, :, :], in_=fin)

    nc.compile()
    return nc


_NC_CACHE = None
LAST_RESULTS = None


def _get_nc():
    global _NC_CACHE
    if _NC_CACHE is None:
        _NC_CACHE = _build_nc()
    return _NC_CACHE


def _host_prep(x, w_comp, b_comp, w_enc, b_enc, power_p):
    """Build per-core input maps (numpy only)."""
    pe = float(np.exp(np.float64(power_p)))

    xc_all = np.ascontiguousarray(
        x.reshape(B, 2, 128, H * W)).astype(NPK)  # [B, 2, 128, HW]

    # slabs [B, 34, 128, 10, C]
    xp = np.pad(x, ((0, 0), (0, 0), (2, 2), (2, 2)))  # [B, C, 132, 132]
    kk = np.arange(-1, 33)
    slabs = np.empty((B, 34, 128, 10, C), dtype=NPX)
    for oh in range(2):
        rows = (2 * kk[:, None] + 64 * np.arange(2)[None, :]) + oh + 2  # [34, 2]
        g0 = xp[:, :, rows, :]                     # [B, C, 34, 2, 132]
        for j in range(KK):
            g = g0[:, :, :, :, j:j + 128:2]        # [B, C, 34, 2, 64]
            slabs[:, :, :, oh * 5 + j, :] = (
                g.transpose(0, 2, 3, 4, 1).reshape(B, 34, 128, C))

    wc = np.ascontiguousarray(
        w_comp[:, :, 0, 0].T.reshape(2, 128, CC)).astype(NPK)
    bc = b_comp.reshape(CC, 1).astype(np.float32)
    wt = np.empty((CC, 9, 25), dtype=NPK)
    for di in range(3):
        for dj in range(3):
            wt[:, 3 * di + dj, :] = (pe * w_enc[:, :, di, dj]).T.astype(NPK)
    be = (pe * b_enc).reshape(25, 1).astype(np.float32)
    idn = np.eye(25, dtype=NPK)
    i128 = np.eye(128, dtype=NPX)

    in_maps = []
    for b in range(B):
        in_maps.append({
            "xc": np.ascontiguousarray(xc_all[b]),
            "slabs": np.ascontiguousarray(slabs[b]),
            "wc": wc, "bc": bc, "wt": wt, "be": be, "idn": idn, "i128": i128,
        })
    return in_maps


def kernel(x, w_comp, b_comp, w_enc, b_enc, power_p):
    x = np.asarray(x, dtype=np.float32)
    in_maps = _host_prep(np.asarray(x), np.asarray(w_comp), np.asarray(b_comp),
                         np.asarray(w_enc), np.asarray(b_enc),
                         np.asarray(power_p))
    nc = _get_nc()
    res = run_bass_kernel_spmd(nc, in_maps, list(range(NCORES)))
    global LAST_RESULTS
    LAST_RESULTS = res
    outs = np.stack([np.asarray(res.results[i]["out"]) for i in range(NCORES)])
    # [B, 32, 128, 256] -> [B, C, 64, 64]; h' = half*32 + k, p = half*64 + w'
    out = (outs.reshape(B, NB, 2, 64, C)
               .transpose(0, 4, 2, 1, 3)
               .reshape(B, C, HP, WP))
    return np.ascontiguousarray(out.astype(np.float32))


# revision 16
# speedup vs baseline: 1.2468x; 1.2468x over previous
"""CARAFE-Downsample Trainium2 kernel (8 NeuronCores, data-parallel over batch).

Problem (hardcoded shapes): x [8, 256, 128, 128] f32; 1x1-conv compressor ->
cx [8, 64, 128, 128]; 3x3 stride-2 conv encoder -> mask [8, 25, 64, 64];
softmax(mask * exp(p)) over the 25 taps; 5x5 stride-2 weighted reassembly of x
-> out [8, 256, 64, 64].

Strategy:
 - one sample per core (B == n_cores == 8).
 - Pixel-block layout: output block k (k in 0..31) holds the 128 output pixels
   {(h', w') : h' in {k, k+32}, w' in 0..63} on the 128 SBUF partitions
   (p = half*64 + w').  With this (k, k+32) row pairing every 5x5 tap of the
   reassembly is a single full-width fused-MAC with the softmax weight as a
   per-partition scalar: tap (i,j) reads host-prepared "slab" (oh, j) at block
   row k + dh (oh = (i-2)%2, dh = (i-2-oh)//2); slabs carry x pre-gathered
   (stride-2 cols, row-parity split) with zero padding baked in (34 block
   rows kk = -1..32).  No transposes of x, no partition shifts, no border
   fix-ups on device.
 - reassembly is split across three engines per block: DVE runs a
   scalar_tensor_tensor chain, ACT produces weighted copies (activation
   Copy with per-partition scale), GPSIMD accumulates those and combines.
 - mask path (compressor + encoder) runs in bf16 on the TensorEngine
   (rel err contribution ~3e-4); softmax weights and the reassembly stay
   f32.  Encoder computes mask [25, 512]-chunks (3x3/s2 conv as 9 matmul
   accumulates with a 2D strided moving operand), then PE-transposes
   [25, 64] slices into the block layout.  exp(power_p) is folded into the
   encoder weights on host; conv biases are K=1 rank-1 matmul accumulates.
 - output is returned in [32 blocks, 128, 256] layout, host restores NCHW.
"""

import numpy as np
import ml_dtypes

import concourse.bass as bass
import concourse.bacc as bacc
import concourse.tile as tile
from concourse import mybir
from concourse.bass_utils import run_bass_kernel_spmd

# -- problem constants (hardcoded per spec) ---------------------------------
B, C, H, W = 8, 256, 128, 128
CC = 64           # compressed channels
KK = 5            # CARAFE window
HP = WP = 64      # output spatial
NB = 32           # pixel blocks per sample
NCORES = 8

# dtype knobs
X_DTYPE = "bf16"    # slabs ("f32" safe / "bf16" fast; PE reassembly needs bf16)
MASK_DTYPE = "bf16"  # compressor/encoder path (error contribution ~3e-4)

# reassembly engine: "pe" = diag-matmul accumulation on TensorE (bf16 slabs,
# f32 psum accumulate, ~2.8e-3 rel err); "dve" = f32 fused-MAC chains split
# across DVE/ACT/GPSIMD (~4e-4 rel err)
REASM = "pe"

# dve-mode tap split: 25 taps total; N_ACT go ACT(product)+GPSIMD(add)
N_ACT = 7
# pe-mode: number of taps run as a DVE bf16 chain instead of PE diag-matmuls
N_DVE = 0

_DTM = {"f32": mybir.dt.float32, "bf16": mybir.dt.bfloat16}
_NPM = {"f32": np.float32, "bf16": ml_dtypes.bfloat16}
DTX, DTK = _DTM[X_DTYPE], _DTM[MASK_DTYPE]
NPX, NPK = _NPM[X_DTYPE], _NPM[MASK_DTYPE]
F32 = mybir.dt.float32

# tap -> (slab index, block-row offset). slab sl = oh*5 + j holds x rows of
# parity oh, cols (j-2)+2*w'' (zero padded), block rows kk = -1..32.
def _tap_table():
    taps = []
    for i in range(KK):
        oh = (i - 2) % 2
        dh = (i - 2 - oh) // 2
        for j in range(KK):
            taps.append((i * 5 + j, oh * 5 + j, dh))
    return taps

_TAPS = _tap_table()


def _build_nc():
    nc = bacc.Bacc(None, target_bir_lowering=False, debug=False)

    xc_d = nc.declare_dram_parameter("xc", [2, 128, H * W], DTK, isOutput=False)
    sl_d = nc.declare_dram_parameter("slabs", [34, 128, 10, C], DTX, isOutput=False)
    wc_d = nc.declare_dram_parameter("wc", [2, 128, CC], DTK, isOutput=False)
    bc_d = nc.declare_dram_parameter("bc", [CC, 1], F32, isOutput=False)
    wt_d = nc.declare_dram_parameter("wt", [CC, 9, 25], DTK, isOutput=False)
    be_d = nc.declare_dram_parameter("be", [25, 1], F32, isOutput=False)
    id_d = nc.declare_dram_parameter("idn", [25, 25], DTK, isOutput=False)
    i128_d = nc.declare_dram_parameter("i128", [128, 128], DTX, isOutput=False)
    out_d = nc.declare_dram_parameter("out", [NB, 128, C], F32, isOutput=True)

    CXW = 130  # padded cx row length; cx_pad[c, r*130 + col], r/col offset by 1

    with tile.TileContext(nc) as tc:
        with (
            tc.tile_pool(name="consts", bufs=1) as consts,
            tc.tile_pool(name="xcin", bufs=3) as xcin,
            tc.tile_pool(name="cx", bufs=1) as cxpool,
            tc.tile_pool(name="psA", bufs=2, space="PSUM") as psA,
            tc.tile_pool(name="psM", bufs=2, space="PSUM") as psM,
            tc.tile_pool(name="psT", bufs=1, space="PSUM") as psT,
            tc.tile_pool(name="psO", bufs=3, space="PSUM") as psO,
            tc.tile_pool(name="soft", bufs=6) as soft,
            tc.tile_pool(name="wmask", bufs=8) as wmask,
            tc.tile_pool(name="slab", bufs=7) as slabp,
            tc.tile_pool(name="accp", bufs=4) as accp,
            tc.tile_pool(name="prod", bufs=8) as prodp,
        ):
            # ---- constants / weights ----
            wc_sb = consts.tile([128, 2, CC], DTK)
            nc.sync.dma_start(out=wc_sb, in_=wc_d[:, :, :].rearrange("c p m -> p c m"))
            wt_sb = consts.tile([CC, 9, 25], DTK)
            nc.sync.dma_start(out=wt_sb, in_=wt_d[:, :, :])
            bc_sb = consts.tile([CC, 1], F32)
            nc.sync.dma_start(out=bc_sb, in_=bc_d[:, :])
            be_sb = consts.tile([25, 1], F32)
            nc.sync.dma_start(out=be_sb, in_=be_d[:, :])
            id_sb = consts.tile([25, 25], DTK)
            nc.sync.dma_start(out=id_sb, in_=id_d[:, :])
            i128_sb = consts.tile([128, 128], DTX)
            nc.sync.dma_start(out=i128_sb, in_=i128_d[:, :])

            # ---- cx_pad (compressor output, 1-px zero ring, flat layout) ----
            cx_pad = cxpool.tile([CC, CXW * CXW], DTK)
            cp = cx_pad[:, :]
            # zero pad row 0 / col 0 (the only pad the encoder reads) via ACT
            # so cx_pad has a single writer engine (keeps PE matmul waits at 1)
            zrow = consts.tile([CC, CXW], DTK)
            nc.vector.memset(zrow, 0.0)
            nc.scalar.copy(out=cp[:, 0:CXW], in_=zrow[:, :])
            nc.scalar.copy(
                out=bass.AP(tensor=cp.tensor, offset=cp.offset + CXW,
                            ap=[cp.ap[0], [CXW, 129], [1, 1]]),
                in_=zrow[:, 0:129],
            )

            # all-engine sync after const loads: keeps every later PE matmul
            # at <=1 sync wait (PE LDWEIGHTS has a single wait slot)
            tc.strict_bb_all_engine_barrier()

            # ---- phase A: compressor 1x1 conv (PE, bf16) ----
            for j in range(32):
                xt = xcin.tile([128, 2, 512], DTK)
                nc.sync.dma_start(
                    out=xt,
                    in_=xc_d[:, :, j * 512:(j + 1) * 512].rearrange("c p n -> p c n"),
                )
                pm = psA.tile([CC, 512], F32)
                nc.tensor.matmul(pm, lhsT=wc_sb[:, 0, :], rhs=xt[:, 0, :],
                                 start=True, stop=False)
                nc.tensor.matmul(pm, lhsT=wc_sb[:, 1, :], rhs=xt[:, 1, :],
                                 start=False, stop=True)
                # rows 4j..4j+3 of cx -> cx_pad interior (offset by 1 row/col)
                dst = bass.AP(tensor=cp.tensor,
                              offset=cp.offset + (4 * j + 1) * CXW + 1,
                              ap=[cp.ap[0], [CXW, 4], [1, 128]])
                nc.scalar.activation(out=dst,
                                     in_=pm[:, :].rearrange("p (r n) -> p r n", n=128),
                                     func=mybir.ActivationFunctionType.Identity,
                                     bias=bc_sb[:, :])

            # ---- phase B: encoder 3x3/s2 conv -> m_all [25, 4096] (bf16) ----
            m_all = cxpool.tile([25, HP * WP], DTK)
            for j2 in range(8):
                pmM = psM.tile([25, 512], F32)
                ti = 0
                for di in range(3):
                    for dj in range(3):
                        # output pixels h' = 8*j2 + r (r 0..7), w' 0..63;
                        # reads cx_pad row 2h'+di, col 2w'+dj
                        rhs = bass.AP(
                            tensor=cp.tensor,
                            offset=cp.offset + (16 * j2 + di) * CXW + dj,
                            ap=[cp.ap[0], [2 * CXW, 8], [2, 64]],
                        )
                        nc.tensor.matmul(pmM, lhsT=wt_sb[:, ti, :], rhs=rhs,
                                         start=(ti == 0), stop=(ti == 8))
                        ti += 1
                nc.scalar.activation(out=m_all[:, j2 * 512:(j2 + 1) * 512],
                                     in_=pmM,
                                     func=mybir.ActivationFunctionType.Identity,
                                     bias=be_sb[:, :])

            # ---- phase C: per block: transpose + exp + softmax weights ----
            w_blocks = []
            for k in range(NB):
                e_k = soft.tile([128, 25], F32)
                for half in range(2):
                    hcol = (k + 32 * half) * 64
                    pmT = psT.tile([64, 25], DTK)
                    nc.tensor.transpose(pmT, m_all[:, hcol:hcol + 64], id_sb[:, :])
                    nc.scalar.activation(out=e_k[half * 64:(half + 1) * 64, :],
                                         in_=pmT,
                                         func=mybir.ActivationFunctionType.Exp)
                r_k = soft.tile([128, 1], F32)
                nc.vector.reduce_sum(out=r_k, in_=e_k, axis=mybir.AxisListType.X)
                nc.vector.reciprocal(out=r_k, in_=r_k)
                w_k = wmask.tile([128, 25], F32)
                rb = bass.AP(tensor=r_k.tensor, offset=r_k.offset,
                             ap=[r_k.ap[0], [0, 25]])
                nc.vector.tensor_tensor(out=w_k, in0=e_k, in1=rb,
                                        op=mybir.AluOpType.mult)
                w_blocks.append(w_k)

            # ---- phase D: reassembly, 3-engine split per block ----
            slab_tiles = []
            for kk in range(34):
                st = slabp.tile([128, 10, C], DTX)
                nc.sync.dma_start(out=st, in_=sl_d[kk, :, :, :])
                slab_tiles.append(st)

            tapmap = {t: (sl, dh) for (t, sl, dh) in _TAPS}
            if REASM == "pe":
                # N_DVE taps run as a DVE bf16 fused-MAC chain; the rest are
                # diag-matmul accumulates on PE: psum += diag(w_t) @ slab_slice
                all_taps = sorted(tapmap)
                dve_taps = all_taps[:N_DVE]
                pe_taps = all_taps[N_DVE:]
                with tc.tile_pool(name="diag", bufs=16) as diagp:
                    for k in range(NB):
                        w_k = w_blocks[k]
                        po = psO.tile([128, C], F32)
                        for n, t in enumerate(pe_taps):
                            sl, dh = tapmap[t]
                            D = diagp.tile([128, 128], DTX, name=f"D_{k}_{t}",
                                           tag="diag")
                            nc.vector.tensor_scalar(out=D, in0=i128_sb,
                                                    scalar1=w_k[:, t:t + 1],
                                                    scalar2=None,
                                                    op0=mybir.AluOpType.mult)
                            nc.tensor.matmul(po, lhsT=D,
                                             rhs=slab_tiles[k + dh + 1][:, sl, :],
                                             start=(n == 0),
                                             stop=(n == len(pe_taps) - 1))
                        fin = accp.tile([128, C], F32, tag="fin")
                        if dve_taps:
                            accs = [accp.tile([128, C], DTX, name=f"acc{i}_{k}",
                                              tag=f"acc{i}") for i in range(2)]
                            for n, t in enumerate(dve_taps):
                                sl, dh = tapmap[t]
                                src_ = slab_tiles[k + dh + 1][:, sl, :]
                                sc = w_k[:, t:t + 1]
                                a = accs[n % 2]
                                if n < 2:
                                    nc.vector.tensor_scalar(out=a, in0=src_,
                                                            scalar1=sc, scalar2=None,
                                                            op0=mybir.AluOpType.mult)
                                else:
                                    nc.vector.scalar_tensor_tensor(
                                        out=a, in0=src_, scalar=sc, in1=a,
                                        op0=mybir.AluOpType.mult,
                                        op1=mybir.AluOpType.add)
                            nc.vector.scalar_tensor_tensor(
                                out=fin, in0=accs[0], scalar=1.0, in1=accs[1],
                                op0=mybir.AluOpType.mult, op1=mybir.AluOpType.add)
                            nc.vector.tensor_tensor(out=fin, in0=fin, in1=po,
                                                    op=mybir.AluOpType.add)
                        else:
                            nc.scalar.copy(out=fin, in_=po)
                        nc.sync.dma_start(out=out_d[k, :, :], in_=fin)
            else:
                # center tap (dh=0) first on DVE to initialize its accumulator;
                # N_ACT taps go to ACT(product) + GPSIMD(accumulate)
                dve_order = [12] + [t for t in range(25) if t != 12][N_ACT:]
                act_order = [t for t in range(25) if t != 12][:N_ACT]
                for k in range(NB):
                    w_k = w_blocks[k]
                    acc = accp.tile([128, C], DTX)
                    fin = accp.tile([128, C], F32, tag="fin")
                    acc2 = accp.tile([128, C], F32, tag="acc2")
                    prods = []
                    for t in act_order:
                        sl, dh = tapmap[t]
                        p_t = prodp.tile([128, C], F32, name=f"p_{k}_{t}", tag="prod")
                        nc.scalar.activation(out=p_t,
                                             in_=slab_tiles[k + dh + 1][:, sl, :],
                                             func=mybir.ActivationFunctionType.Copy,
                                             scale=w_k[:, t:t + 1])
                        prods.append(p_t)
                    nc.gpsimd.tensor_add(acc2, prods[0], prods[1])
                    for p_t in prods[2:]:
                        nc.gpsimd.tensor_add(acc2, acc2, p_t)
                    for n, t in enumerate(dve_order):
                        sl, dh = tapmap[t]
                        src_ = slab_tiles[k + dh + 1][:, sl, :]
                        sc = w_k[:, t:t + 1]
                        if n == 0:
                            nc.vector.tensor_scalar(out=acc, in0=src_, scalar1=sc,
                                                    scalar2=None,
                                                    op0=mybir.AluOpType.mult)
                        else:
                            nc.vector.scalar_tensor_tensor(
                                out=acc, in0=src_, scalar=sc, in1=acc,
                                op0=mybir.AluOpType.mult, op1=mybir.AluOpType.add)
                    nc.gpsimd.tensor_add(fin, acc, acc2)
                    nc.sync.dma_start(out=out_d[k, :, :], in_=fin)

    nc.compile()
    return nc


_NC_CACHE = None
LAST_RESULTS = None


def _get_nc():
    global _NC_CACHE
    if _NC_CACHE is None:
        _NC_CACHE = _build_nc()
    return _NC_CACHE


def _host_prep(x, w_comp, b_comp, w_enc, b_enc, power_p):
    """Build per-core input maps (numpy only)."""
    pe = float(np.exp(np.float64(power_p)))

    xc_all = np.ascontiguousarray(
        x.reshape(B, 2, 128, H * W)).astype(NPK)  # [B, 2, 128, HW]

    # slabs [B, 34, 128, 10, C]
    xp = np.pad(x, ((0, 0), (0, 0), (2, 2), (2, 2)))  # [B, C, 132, 132]
    kk = np.arange(-1, 33)
    slabs = np.empty((B, 34, 128, 10, C), dtype=NPX)
    for oh in range(2):
        rows = (2 * kk[:, None] + 64 * np.arange(2)[None, :]) + oh + 2  # [34, 2]
        g0 = xp[:, :, rows, :]                     # [B, C, 34, 2, 132]
        for j in range(KK):
            g = g0[:, :, :, :, j:j + 128:2]        # [B, C, 34, 2, 64]
            slabs[:, :, :, oh * 5 + j, :] = (
                g.transpose(0, 2, 3, 4, 1).reshape(B, 34, 128, C))

    wc = np.ascontiguousarray(
        w_comp[:, :, 0, 0].T.reshape(2, 128, CC)).astype(NPK)
    bc = b_comp.reshape(CC, 1).astype(np.float32)
    wt = np.empty((CC, 9, 25), dtype=NPK)
    for di in range(3):
        for dj in range(3):
            wt[:, 3 * di + dj, :] = (pe * w_enc[:, :, di, dj]).T.astype(NPK)
    be = (pe * b_enc).reshape(25, 1).astype(np.float32)
    idn = np.eye(25, dtype=NPK)
    i128 = np.eye(128, dtype=NPX)

    in_maps = []
    for b in range(B):
        in_maps.append({
            "xc": np.ascontiguousarray(xc_all[b]),
            "slabs": np.ascontiguousarray(slabs[b]),
            "wc": wc, "bc": bc, "wt": wt, "be": be, "idn": idn, "i128": i128,
        })
    return in_maps


def kernel(x, w_comp, b_comp, w_enc, b_enc, power_p):
    x = np.asarray(x, dtype=np.float32)
    in_maps = _host_prep(np.asarray(x), np.asarray(w_comp), np.asarray(b_comp),
                         np.asarray(w_enc), np.asarray(b_enc),
                         np.asarray(power_p))
    nc = _get_nc()
    res = run_bass_kernel_spmd(nc, in_maps, list(range(NCORES)))
    global LAST_RESULTS
    LAST_RESULTS = res
    outs = np.stack([np.asarray(res.results[i]["out"]) for i in range(NCORES)])
    # [B, 32, 128, 256] -> [B, C, 64, 64]; h' = half*32 + k, p = half*64 + w'
    out = (outs.reshape(B, NB, 2, 64, C)
               .transpose(0, 4, 2, 1, 3)
               .reshape(B, C, HP, WP))
    return np.ascontiguousarray(out.astype(np.float32))
